# revision 19
# baseline (speedup 1.0000x reference)
"""MAMDense Trainium2 kernel (fp16 2x-packed DVE).

C[m, n] = max_k(x[m,k] * W[n,k]) + min_k(x[m,k] * W[n,k]) + bias[n]
x: [2048, 1024] f32, W: [1024, 1024] f32, bias: [1024] f32 -> C: [2048, 1024] f32

Strategy (data parallel over M, 8 cores, 256 rows each):
- W blocks live in SBUF as fp16 ([128 n-rows, NB*K]); x rows are broadcast
  across partitions via PE (row-extract matmul + ones-matmul) and cast to
  fp16 by the ACT engine (PSUM fp32 -> SBUF fp16).
- The core op is a custom DVE instruction running in 2X_1PORT perf mode:
  each cycle reads one 32-bit word from each source (two packed fp16
  elements), forms both products, folds them into four fp32 running
  accumulators (even/odd max, even/odd min) held in stage flops, and emits
  the combined running (max, min) pair packed into one 32-bit write.
  A 3-state uop FSM (consuming reseed / steady / reseed-step) re-seeds the
  accumulators at every SUB_DIM_DONE page boundary, so one instruction
  covers all 8 n-blocks for one m row. 2 elems/cycle/partition = 2x fp32.
- The last packed word of each page holds (max_final, min_final); a small
  1x DVE add extracts ctT[n, m] = max + min into fp32.
- Endgame: add bias, PE-transpose C^T back to [M, N], DMA out.
"""
import numpy as np

M, K, N = 2048, 1024, 1024
N_CORES = 8
M_LOC = M // N_CORES  # 256
P = 128
NB = N // P           # 8 n-blocks
NT = M_LOC // P       # 2 m-tiles
FMAX = 3.4028234663852886e38

_STATE = {}


def _build_mam2x_uops(final_only=False):
    """Hand-built 3-state uop program for the packed 2x MAM op.

    Per 32-bit word (elements 2t, 2t+1) the steady state computes
      p_lo = w_lo*x_lo, p_hi = w_hi*x_hi
      accE = max(accE, p_lo); accO = max(accO, p_hi)   (stage-flop fp32)
      minE = min(minE, p_lo); minO = min(minO, p_hi)
      M = max(accE, accO); MM = min(minE, minO)
    and writes (M, MM) packed to WR0_LO/WR0_HI. The reseed state (entered
    at instruction start and at each SUB_DIM_DONE page boundary) CONSUMES
    the first word of the page, substituting +-FLT_MAX for the stale
    accumulators — the same consuming-initialize structure the stock 2x
    paged-mask program uses (no non-consuming bubble states)."""
    from concourse.dve_uop import (
        UopConfig, InpSel, OutSel, OutPath, AluOp, AluInp,
        DelayInp, Trigger, ENABLE,
    )

    PD = AluInp.PREV_DELAY_0
    PREV = AluInp.PREV_ALU_OUT
    CURR = AluInp.CURR_ALU_OUT
    CAP = DelayInp.PREV_ALU_OUT

    def base_inputs(u):
        u.enable_input(InpSel.SRC_0, 0)      # w_lo -> stage-0 ALU
        u.enable_input(InpSel.SRC_1, 1)      # x_lo -> delay_0
        u.enable_input(InpSel.SRC_0_HI, 4)   # w_hi -> delay_3
        u.enable_input(InpSel.SRC_1_HI, 5)   # x_hi -> delay_4
        u.require_inp0 = ENABLE
        u.require_inp1 = ENABLE
        return u

    def make_reseed():
        """Consuming page-initialize: accs = f(+-FLT_MAX, products)."""
        u = base_inputs(UopConfig())
        u.enable_input(InpSel.MAX_NEG, 2)    # -> delay_1
        u.enable_input(InpSel.MAX_POS, 3)    # -> delay_2
        dp = u.datapath_config
        dp[0].enable_alu(AluOp.MULTIPLY, PREV, AluInp(PD + 0)) \
            .pass_through_delay(1, 2, 3, 4)                       # p_lo
        dp[1].enable_alu(AluOp.MULTIPLY, AluInp(PD + 3), AluInp(PD + 4)) \
            .enable_delay_from_src(CAP, 0).pass_through_delay(1, 2)  # d0<-p_lo
        dp[2].enable_alu(AluOp.MAX, AluInp(PD + 1), AluInp(PD + 0)) \
            .enable_delay_from_src(CAP, 5) \
            .pass_through_delay(0, 1, 2)                          # d5<-p_hi
        dp[3].enable_alu(AluOp.MAX, AluInp(PD + 1), AluInp(PD + 5)) \
            .enable_delay_from_src(CAP, 3) \
            .pass_through_delay(0, 2, 5)                          # d3<-accE
        dp[4].enable_alu(AluOp.MIN, AluInp(PD + 2), AluInp(PD + 0)) \
            .enable_delay_from_src(CAP, 4) \
            .pass_through_delay(2, 3, 5)                          # d4<-accO
        dp[5].enable_alu(AluOp.MIN, AluInp(PD + 2), AluInp(PD + 5)) \
            .enable_delay_from_src(CAP, 0) \
            .pass_through_delay(3, 4)                             # d0<-minE
        dp[6].enable_alu(AluOp.MAX, AluInp(PD + 3), AluInp(PD + 4)) \
            .enable_delay_from_src(CAP, 1) \
            .pass_through_delay(0)                                # d1<-minO
        dp[7].enable_alu(AluOp.MIN, AluInp(PD + 0), AluInp(PD + 1)) \
            .enable_delay_from_src(CAP, 2)                        # d2<-M
        u.enable_output(OutSel.DELAY_2, OutPath.WR0_LO)           # M
        u.enable_output(OutSel.ALU_OUT, OutPath.WR0_HI)           # MM
        u.out_last_subdim_enable = ENABLE if final_only else 0
        u.repeat_count = 1
        u.trigger = (Trigger.SRC_TENSOR_DONE, Trigger.SUB_DIM_DONE,
                     Trigger.COUNT)
        u.next_uop = (0, 2, 1)
        return u

    steady = base_inputs(UopConfig())
    dp = steady.datapath_config
    dp[0].enable_alu(AluOp.MULTIPLY, PREV, AluInp(PD + 0)) \
        .pass_through_delay(3, 4)
    dp[1].enable_alu(AluOp.MULTIPLY, AluInp(PD + 3), AluInp(PD + 4)) \
        .enable_delay_from_src(CAP, 0)                            # d0 <- p_lo
    dp[2].enable_alu(AluOp.MAX, CURR, AluInp(PD + 0)) \
        .enable_delay_from_src(CAP, 1) \
        .pass_through_delay(0)                                    # d1 <- p_hi
    dp[3].enable_alu(AluOp.MAX, CURR, AluInp(PD + 1)) \
        .enable_delay_from_src(CAP, 2) \
        .pass_through_delay(0, 1)                                 # d2 <- accE
    dp[4].enable_alu(AluOp.MIN, CURR, AluInp(PD + 0)) \
        .enable_delay_from_src(CAP, 3) \
        .pass_through_delay(1, 2)                                 # d3 <- accO
    dp[5].enable_alu(AluOp.MIN, CURR, AluInp(PD + 1)) \
        .enable_delay_from_src(CAP, 4) \
        .pass_through_delay(2, 3)                                 # d4 <- minE
    dp[6].enable_alu(AluOp.MAX, AluInp(PD + 2), AluInp(PD + 3)) \
        .enable_delay_from_src(CAP, 5) \
        .pass_through_delay(4)                                    # d5 <- minO
    dp[7].enable_alu(AluOp.MIN, AluInp(PD + 4), AluInp(PD + 5)) \
        .enable_delay_from_src(CAP, 0)                            # d0 <- M
    steady.enable_output(OutSel.DELAY_0, OutPath.WR0_LO)          # M
    steady.enable_output(OutSel.ALU_OUT, OutPath.WR0_HI)          # MM
    steady.out_last_subdim_enable = ENABLE if final_only else 0
    steady.trigger = (Trigger.SRC_TENSOR_DONE, Trigger.SUB_DIM_DONE,
                      Trigger.NONE)
    steady.next_uop = (0, 2, 0)

    return [make_reseed(), steady, make_reseed()]


def _build_mam_1x_safe_uops():
    """Guaranteed-terminating 1x fallback (3 states to match the 2x variant).
    Only runs if the RTL declines 2x mode; emits products (wrong results,
    caught by the rel-err check) but always consumes and exits."""
    from concourse.dve_uop import (
        UopConfig, InpSel, OutSel, OutPath, AluOp, AluInp, Trigger, ENABLE,
    )

    def make():
        u = UopConfig()
        u.enable_input(InpSel.SRC_0, 0)
        u.enable_input(InpSel.SRC_1, 1)
        u.require_inp0 = ENABLE
        u.require_inp1 = ENABLE
        u.datapath_config[0].enable_alu(
            AluOp.MULTIPLY, AluInp.PREV_ALU_OUT, AluInp.PREV_DELAY_0)
        for i in range(1, 8):
            u.datapath_config[i].pass_through_alu()
        u.enable_output(OutSel.ALU_OUT, OutPath.WR0_LO)
        u.trigger = (Trigger.SRC_TENSOR_DONE, Trigger.SUB_DIM_DONE,
                     Trigger.NONE)
        u.next_uop = (0, 2, 0)
        return u

    return [make(), make(), make()]


def _register_mam2x_op(final_only=False):
    """Register the packed-2x multiply->paged-(max,min) DVE op.

    final_only=True sets write_subdim_last on every state: only each page's
    final packed (max, min) word is written, so the dst is [P, S, 2] and no
    per-row extraction pass is needed."""
    import concourse.dve_ops as dve_ops
    from concourse.dve_ops import DveOp
    from concourse.dve_spec import Spec, Src0, Src1, C0, scan, AluOp
    from concourse.dve_uop import DveOpSpec

    name = "MAM2XF_PAGED_ANT" if final_only else "MAM2X_PAGED_ANT"
    for op in dve_ops.OPS:
        if op.name == name:
            return op

    def _reference(in0, in1, s0, s1, imm2):
        # 2x-packed semantics: word t's lo half = running max after element
        # 2t+1, hi half = running min after element 2t+1 (per page).
        prod = in0 * in1
        mx = np.maximum.accumulate(prod, axis=-1)
        mn = np.minimum.accumulate(prod, axis=-1)
        if final_only:
            return np.stack([mx[..., -1], mn[..., -1]], axis=-1)
        out = np.empty_like(prod)
        out[..., 0::2] = mx[..., 1::2]
        out[..., 1::2] = mn[..., 1::2]
        return out

    prod = Src0 * Src1
    spec = Spec(
        body=scan(AluOp.MAX, prod) + scan(AluOp.MIN, prod, init=C0),
        reference=_reference,
    )

    uops_2x = _build_mam2x_uops(final_only)
    uops_1x = _build_mam_1x_safe_uops()
    row = dve_ops._CUSTOM_DVE_ROW_BASE + len(dve_ops.OPS)
    shas, compiled = {}, {}
    for ver in ("v3", "v4"):
        try:
            s = DveOpSpec(name=name, opcode=row, uops=uops_1x,
                          uops_2x=uops_2x, rd1_en=True, perf_max=1)
            s.validate(ver)
            compiled[ver] = s
            shas[ver] = s.sha(ver)
        except Exception:
            pass
    assert "v3" in compiled, "2x uop program failed validation for v3"
    op = DveOp(name, spec, subdim=True, uops_sha=shas)
    dve_ops.OPS.append(op)
    dve_ops._SUB_OPCODE_FOR_NAME[name] = row
    dve_ops.CUSTOM_DVE_SPECS[name] = spec
    for ver, s in compiled.items():
        dve_ops._COMPILE_CACHE[(name, ver)] = s
    return op


def _emit_mam2x(nc, op, out, in0, in1):
    """Emit the custom DVE instruction with perf_max=1 (2X_1PORT reachable).
    Mirrors bass.BassVectorEngine._custom_dve, which hardcodes perf_max=0."""
    import concourse.mybir as mybir
    from concourse import bass_isa
    from concourse.dve_ops import get_dve_sub_opcode

    v = nc.vector
    if op.name not in v.bass.m.ant_custom_dve_ops:
        v.bass.m.ant_custom_dve_ops = sorted(
            {*v.bass.m.ant_custom_dve_ops, op.name})
    shape = bass_isa.CustomDveShape.STT  # in1 has 2 free dims
    isa_opcode = v.bass.isa.Opcode[
        f"NEURON_ISA_TPB_OPCODE_CUSTOM_DVE_ANT_{shape.slot()}"].value
    zero = mybir.ImmediateValue(dtype=mybir.dt.float32, value=0.0)
    ins = [v.lower_ap(in0, for_isa=True, opt=False),
           v.lower_ap(in1, for_isa=True, opt=False), zero, zero]
    outs = [v.lower_ap(out, for_isa=True, opt=False)]
    return v.add_instruction(
        bass_isa.InstCustomDveAnt(
            name=v.bass.get_next_instruction_name(),
            op_name=op.name,
            rd1_en=True,
            subdim=0x02,
            imm2=0.0,
            shape=shape,
            row=get_dve_sub_opcode(op.name),
            isa_opcode=isa_opcode,
            perf_max=1,
            ins=ins,
            outs=outs,
        )
    )


def build_nc(replicas: int = 1):
    """Build + compile the per-core Bacc program. `replicas` repeats the
    compute body (for timing-by-differencing in test harnesses)."""
    import concourse.bacc as bacc
    import concourse.mybir as mybir
    from concourse.tile import TileContext

    MAM2X = _register_mam2x_op(final_only=True)
    f32, f16 = mybir.dt.float32, mybir.dt.float16

    nc = bacc.Bacc("TRN2", target_bir_lowering=False, debug=False)
    x = nc.dram_tensor("x", [M_LOC, K], f32, kind="ExternalInput")
    w = nc.dram_tensor("weight", [N, K], f16, kind="ExternalInput")
    b = nc.dram_tensor("bias", [P, NB], f32, kind="ExternalInput")
    ident = nc.dram_tensor("ident", [P, P], f32, kind="ExternalInput")
    out = nc.dram_tensor("out", [M_LOC, N], f32, kind="ExternalOutput")

    with TileContext(nc) as tc:
        with tc.tile_pool(name="const", bufs=1) as cpool, \
             tc.tile_pool(name="psum", bufs=2, space="PSUM") as ppool, \
             tc.tile_pool(name="stage", bufs=4) as spool, \
             tc.tile_pool(name="xrow", bufs=4) as xpool:
            # --- loads -----------------------------------------------------
            xt = []
            for t in range(NT):
                xti = cpool.tile([P, K], f32, name=f"xt{t}", tag=f"xt{t}")
                nc.sync.dma_start(out=xti[:], in_=x.ap()[t * P:(t + 1) * P, :])
                xt.append(xti)
            bias_pb = cpool.tile([P, NB], f32, tag="bias_pb")
            nc.sync.dma_start(out=bias_pb[:], in_=b.ap()[:, :])
            ident_t = cpool.tile([P, P], f32, tag="ident_t")
            nc.sync.dma_start(out=ident_t[:], in_=ident.ap()[:, :])
            ones_all = cpool.tile([P, P], f32, tag="ones_all")
            nc.gpsimd.memset(ones_all[:], 1.0)

            # fp16 weight (cast on host, DMAed directly)
            w16 = cpool.tile([P, NB * K], f16, tag="w16")
            for blk in range(NB):
                nc.sync.dma_start(out=w16[:, blk * K:(blk + 1) * K],
                                  in_=w.ap()[blk * P:(blk + 1) * P, :])
            w16_3d = w16[:].rearrange("p (s n) -> p s n", s=NB)

            # res holds each row's 8 page-final packed (max, min) pairs —
            # the MAM op writes only the final word per page
            # (write_subdim_last), so no per-row extraction pass exists
            res = cpool.tile([P, M_LOC * NB * 2], f16, tag="res")
            resv = res[:].rearrange("p (m q) -> p m q", m=M_LOC)

            ctT = cpool.tile([P, NB * M_LOC], f32, tag="ctT")
            ct = [cpool.tile([P, N], f32, name=f"ct{t}", tag=f"ct{t}")
                  for t in range(NT)]
            ctT3d = ctT[:].rearrange("p (s m) -> p s m", s=NB)

            # --- main loop: groups of 4 consecutive x rows -------------------
            # Rows are extracted 4 at a time into PSUM partition quadrants
            # {0,32,64,96} (PE tile_position), one ACT copy serves 4 rows,
            # then a per-quadrant ones-matmul broadcasts each row across all
            # 128 partitions; ACT casts it to fp16 in SBUF for the DVE.
            NG = M_LOC // 4

            def emit_extract(g):
                """PE-A row extract x4 into PSUM quadrants + one ACT copy."""
                pe4 = ppool.tile([P, K], f32, name=f"pe4_{g}", tag="pe4")
                for j in range(4):
                    m = 4 * g + j
                    t, r = divmod(m, P)
                    q = 32 * j
                    sel = ident_t[:, r:r + 1]
                    for h in range(K // 512):
                        nc.tensor.matmul(
                            pe4[q:q + 1, h * 512:(h + 1) * 512], sel,
                            xt[t][:, h * 512:(h + 1) * 512],
                            start=True, stop=True, tile_position=(0, q))
                st4 = spool.tile([P, K], f32, name=f"st4_{g}", tag="st4")
                nc.scalar.copy(st4[:], pe4[:])
                return st4

            for _ in range(replicas):
                st4q = [emit_extract(0), emit_extract(1)]
                for g in range(NG):
                    st4_cur = st4q.pop(0)
                    if g + 2 < NG:
                        st4q.append(emit_extract(g + 2))
                    for j in range(4):
                        m = 4 * g + j
                        q = 32 * j
                        xps = ppool.tile([P, K], f32, tag="xps")
                        for h in range(K // 512):
                            nc.tensor.matmul(
                                xps[:, h * 512:(h + 1) * 512],
                                ones_all[q:q + 1, :],
                                st4_cur[q:q + 1, h * 512:(h + 1) * 512],
                                start=True, stop=True, tile_position=(q, 0))
                        x16 = xpool.tile([P, K], f16, tag="x16")
                        nc.scalar.copy(x16[:], xps[:])
                        _emit_mam2x(
                            nc, MAM2X,
                            out=res[:, m * 2 * NB:(m + 1) * 2 * NB]
                                .rearrange("p (s t) -> p s t", s=NB),
                            in0=w16_3d,
                            in1=x16[:].unsqueeze(1).broadcast_to([P, NB, K]),
                        )

            # --- endgame: C^T = max + min (one bulk add), bias, transpose ----
            ctT3 = ctT[:].rearrange("p (s m) -> p m s", s=NB)
            nc.vector.tensor_add(ctT3, resv[:, :, 0::2], resv[:, :, 1::2])
            for blk in range(NB):
                nc.vector.tensor_scalar_add(
                    ctT[:, blk * M_LOC:(blk + 1) * M_LOC],
                    ctT[:, blk * M_LOC:(blk + 1) * M_LOC],
                    bias_pb[:, blk:blk + 1])
            for blk in range(NB):
                for t in range(NT):
                    tp = ppool.tile([P, P], f32, tag="xps")
                    nc.tensor.transpose(
                        tp[:], ctT[:, blk * M_LOC + t * P:blk * M_LOC + (t + 1) * P],
                        ident_t[:])
                    nc.scalar.copy(ct[t][:, blk * P:(blk + 1) * P], tp[:])
            for t in range(NT):
                nc.sync.dma_start(out=out.ap()[t * P:(t + 1) * P, :],
                                  in_=ct[t][:])
    nc.compile()
    return nc


def _get_runner(replicas: int = 1):
    key = ("runner", replicas)
    if key not in _STATE:
        import jax
        import numpy as _np
        from jax.sharding import Mesh, PartitionSpec
        from jax.experimental.shard_map import shard_map
        import concourse.mybir as mybir
        from concourse import bass2jax
        from concourse.bass2jax import _bass_exec_p, install_neuronx_cc_hook

        install_neuronx_cc_hook()
        nc = build_nc(replicas)

        partition_name = (nc.partition_id_tensor.name
                          if nc.partition_id_tensor else None)
        in_names, out_names, out_avals, zero_shapes = [], [], [], []
        for alloc in nc.m.functions[0].allocations:
            if not isinstance(alloc, mybir.MemoryLocationSet):
                continue
            nm = alloc.memorylocations[0].name
            if alloc.kind == "ExternalInput":
                if nm != partition_name:
                    in_names.append(nm)
            elif alloc.kind == "ExternalOutput":
                out_names.append(nm)
                shape = tuple(alloc.tensor_shape)
                dtype = mybir.dt.np(alloc.dtype)
                out_avals.append(jax.core.ShapedArray(shape, dtype))
                zero_shapes.append((shape, dtype))
        all_in_names = list(in_names) + out_names
        if partition_name is not None:
            all_in_names.append(partition_name)

        def _body(*args):
            operands = list(args)
            if partition_name is not None:
                operands.append(bass2jax.partition_id_tensor())
            outs = _bass_exec_p.bind(
                *operands,
                out_avals=tuple(out_avals),
                in_names=tuple(all_in_names),
                out_names=tuple(out_names),
                lowering_input_output_aliases=(),
                sim_require_finite=True,
                sim_require_nnan=True,
                nc=nc,
            )
            return tuple(outs)

        devices = jax.devices()[:N_CORES]
        mesh = Mesh(_np.asarray(devices), ("core",))
        n_io = len(in_names) + len(out_names)
        fn = jax.jit(
            shard_map(_body, mesh=mesh,
                      in_specs=(PartitionSpec("core"),) * n_io,
                      out_specs=(PartitionSpec("core"),) * len(out_names),
                      check_rep=False),
            keep_unused=True,
        )
        _STATE[key] = (fn, in_names, out_names, out_avals, zero_shapes, mesh)
    return _STATE[key]


def _prepare(x, weight, bias, replicas=1):
    """device_put sharded inputs; returns a zero-arg callable running the
    kernel on device plus metadata for decoding outputs."""
    import jax
    from jax.sharding import NamedSharding, PartitionSpec
    fn, in_names, out_names, out_avals, zero_shapes, mesh = _get_runner(replicas)
    eye = np.eye(P, dtype=np.float32)
    bias_pb = np.ascontiguousarray(bias.reshape(NB, P).T)  # [128, 8]
    w16 = np.ascontiguousarray(weight.astype(np.float16))
    per_core = {
        "x": [x[c * M_LOC:(c + 1) * M_LOC] for c in range(N_CORES)],
        "weight": [w16] * N_CORES,
        "bias": [bias_pb] * N_CORES,
        "ident": [eye] * N_CORES,
    }
    concat_in = [np.concatenate(per_core[nm], axis=0) for nm in in_names]
    concat_zeros = [np.zeros((N_CORES * s[0], *s[1:]), d)
                    for (s, d) in zero_shapes]
    sharding = NamedSharding(mesh, PartitionSpec("core"))
    dev_in = [jax.device_put(a, sharding) for a in concat_in]
    dev_zero = [jax.device_put(a, sharding) for a in concat_zeros]

    def run():
        outs = fn(*dev_in, *dev_zero)
        jax.block_until_ready(outs)
        return outs

    return run, out_names, out_avals


def _run_sharded(x, weight, bias, replicas=1):
    run, out_names, out_avals = _prepare(x, weight, bias, replicas)
    return run(), out_names, out_avals


def kernel(x: np.ndarray, weight: np.ndarray, bias: np.ndarray) -> np.ndarray:
    x = np.ascontiguousarray(np.asarray(x, dtype=np.float32))
    weight = np.ascontiguousarray(np.asarray(weight, dtype=np.float32))
    bias = np.ascontiguousarray(np.asarray(bias, dtype=np.float32))
    outs, out_names, out_avals = _run_sharded(x, weight, bias, replicas=1)
    i = out_names.index("out")
    full = np.asarray(outs[i]).reshape(N_CORES * M_LOC, N)
    return full
